# revision 1
# baseline (speedup 1.0000x reference)
"""Trainium2 Bass kernel for the DGCNN layer (KNN-16 + edge MLP + max pool).

Sharding: data-parallel over batch (B=4) x query-halves (2 per batch) = 8
cores.  Each core holds all N=4096 points of its batch and computes KNN +
MLP + max for its 2048 queries.

Device pipeline per 128-query tile:
  PE    : s = 2*xi.xj - |xj|^2 (fp32, K=4)   -> PSUM
  ACT   : evict s PSUM->SBUF
  DMA   : spill s SBUF->DRAM (gather source)
  GPSIMD: chunk-max tree (chunk=16) -> 256 chunk maxes
  DVE   : top-16 chunks (max/max_index/match_replace)
  DMA   : indirect gather of the 16 chunks -> 256 candidates
  DVE   : exact top-16 of candidates (+ tie-gap flags)
  DMA   : chunk-id lookup round trip; per-coordinate neighbor gathers
  PE    : MLP1 (K=6, f32r) ; ACT: silu+b1 ; PE: MLP2 (K=192, f32r)
  DVE   : max over 16-neighbor groups, +b2 ; DMA: store

Boundary ties (16th vs 17th neighbor closer than fp32 Gram noise) are
flagged on device (elem-gap and chunk-gap) and those few queries are
recomputed on host with reference-identical arithmetic.
"""

from contextlib import ExitStack

import numpy as np

import concourse.bacc as bacc
import concourse.bass as bass
import concourse.mybir as mybir
import concourse.tile as tile
from concourse.bass import IndirectOffsetOnAxis
from concourse.bass_utils import run_bass_kernel_spmd

F32 = mybir.dt.float32
F32R = mybir.dt.float32r
F16 = mybir.dt.float16
U16 = mybir.dt.uint16
U32 = mybir.dt.uint32

B = 4
N = 4096
Q = 2048          # queries per core
NT = Q // 128     # query tiles per core
K = 16
H = 192
C = 384
CHUNK = 16
NCHUNK = N // CHUNK

TIE_EPS = 2e-5    # flag threshold on s-gap (abs); s noise is ~1e-6

Alu = mybir.AluOpType
Act = mybir.ActivationFunctionType
Axis = mybir.AxisListType


def build_program(nc: bass.Bass, mlp_dtype=F32R, gram_dtype=F32, nt=NT):
    ptT2 = nc.dram_tensor("ptT2", [3, N], F16, kind="ExternalInput")
    ptp4 = nc.dram_tensor("ptp4", [N, 4], F16, kind="ExternalInput")
    ident = nc.dram_tensor("ident", [128, 128], F16, kind="ExternalInput")
    qrn = nc.dram_tensor("qrn", [4, Q + N], F32, kind="ExternalInput")
    w1c = nc.dram_tensor("w1c", [6, H], F16, kind="ExternalInput")
    w2 = nc.dram_tensor("w2", [H, C], F16, kind="ExternalInput")
    b1d = nc.dram_tensor("b1d", [H, 1], F32, kind="ExternalInput")
    iota_rb = nc.dram_tensor("iota_rb", [128, K], U32, kind="ExternalInput")
    iota16 = nc.dram_tensor("iota16", [128, K], U32, kind="ExternalInput")

    outT = nc.dram_tensor("outT", [C, Q], F32, kind="ExternalOutput")
    flags = nc.dram_tensor("flags", [128, 2 * NT], F32, kind="ExternalOutput")

    s_dram = nc.dram_tensor("s_dram", [Q, N], F32, kind="Internal")
    j_dbg = nc.dram_tensor("j_dbg", [Q, K], U32, kind="Internal")
    cid_dram = nc.dram_tensor("cid_dram", [Q, K], U16, kind="Internal")

    # 2-D APs with indirection on axis=1 => coef = 1 (flat element offsets)
    s_flat = s_dram[:, :]
    cid_flat = cid_dram[:, :]

    with tile.TileContext(nc) as tc, ExitStack() as ctx:
        pers = ctx.enter_context(tc.tile_pool(name="pers", bufs=1))
        sp_scp = ctx.enter_context(tc.tile_pool(name="scp", bufs=2))
        sp_m = ctx.enter_context(tc.tile_pool(name="m", bufs=2))
        sp_small = ctx.enter_context(tc.tile_pool(name="small", bufs=2))
        sp_rhs = ctx.enter_context(tc.tile_pool(name="rhs", bufs=2))
        sp_s1 = ctx.enter_context(tc.tile_pool(name="s1", bufs=3))
        sp_out = ctx.enter_context(tc.tile_pool(name="osb", bufs=2))
        pspers = ctx.enter_context(
            tc.tile_pool(name="pspers", bufs=1, space=bass.MemorySpace.PSUM))

        qrn_sb = pers.tile([4, Q + N], F32)  # [x;y;z;1 | 2x;2y;2z;-|x|^2]
        qT4 = qrn_sb[:, 0:Q]
        rhs4 = qrn_sb[:, Q:Q + N]
        w1xi = pers.tile([3, H], F16)
        w1xj = pers.tile([3, H], F16)
        w2a = pers.tile([128, C], F16)
        w2b = pers.tile([64, C], F16)
        b1a = pers.tile([128, 1], F32)
        b1b = pers.tile([64, 1], F32)
        io_rb = pers.tile([128, K], U32)
        idsb = pers.tile([128, 128], F16)
        io_16 = pers.tile([128, K], U32)
        flg = pers.tile([128, 2 * NT], F32)
        nc.gpsimd.memset(flg[:, :], 0.0)

        # 1-wait setup dummies: absorb each input-load semaphore on the PE
        # before any fp32/f32r matmul (those accept at most one sync wait).
        # static PSUM (manual ping-pong; same-engine WAW is pipeline-ordered)
        pg2 = [pspers.tile([128, 1024], F32, name="pg0", tag="pg0")]
        p1a = pspers.tile([128, 1024], F32, tag="p1a")
        p1b = pspers.tile([64, 1024], F32, tag="p1b")
        p22 = [pspers.tile([128, 512], F32, name="p2_0", tag="p2_0")]
        pT = pspers.tile([4, 128], F16, tag="pT")

        nc.sync.dma_start(qrn_sb[:, :], qrn[:, :])
        nc.sync.dma_start(w1xi[:, :], w1c[0:3, :])
        nc.sync.dma_start(w1xj[:, :], w1c[3:6, :])
        nc.sync.dma_start(w2a[:, :], w2[0:128, :])
        nc.sync.dma_start(w2b[:, :], w2[128:H, :])
        nc.sync.dma_start(b1a[:, :], b1d[0:128, :])
        nc.sync.dma_start(b1b[:, :], b1d[128:H, :])
        nc.sync.dma_start(io_rb[:, :], iota_rb[:, :])
        nc.sync.dma_start(idsb[:, :], ident[:, :])
        nc.sync.dma_start(io_16[:, :], iota16[:, :])

        setup_dum = [
            (qrn_sb[0:2, 0:2].bitcast(F16), qrn_sb[0:2, 0:2].bitcast(F16)),
            (w1xi[0:2, 0:2], w1xi[0:2, 0:2]),
            (w1xj[0:2, 0:2], w1xj[0:2, 0:2]),
            (w2a[0:2, 0:2], w2a[0:2, 0:2]),
            (w2b[0:2, 0:2], w2b[0:2, 0:2]),
        ]
        for lhs_d, rhs_d in setup_dum:
            nc.tensor.matmul(pg2[0][0:lhs_d.free_size(), 0:rhs_d.free_size()],
                             lhs_d, rhs_d,
                             start=True, stop=True, skip_group_check=True)

        for t in range(nt):
            qs = t * 128

            scp = sp_scp.tile([128, N], F32, tag="scp")
            # tiny ACT op claims the scp slot (waits Pool+DMA readers of the
            # previous occupant) so the real evicts keep within 2 waits.
            nc.scalar.copy(scp[0:1, 0:1], scp[0:1, 0:1])
            for qtr in range(4):
                pg = pg2[0]
                for half in range(2):
                    fs = qtr * 1024 + half * 512
                    nc.tensor.matmul(
                        pg[:, half * 512:(half + 1) * 512],
                        qT4[:, qs:qs + 128].bitcast(gram_dtype),
                        rhs4[:, fs:fs + 512].bitcast(gram_dtype),
                        start=True, stop=True, skip_group_check=True)
                nc.scalar.copy(scp[:, qtr * 1024:(qtr + 1) * 1024], pg[:, :])
            nc.sync.dma_start(s_dram[qs:qs + 128, :], scp[:, :])

            # chunk-max (DVE windowed reduce; Pool cannot run TensorTensor)
            M = sp_m.tile([128, NCHUNK], F32, tag="M")
            nc.vector.reduce_max(
                M[:, :],
                scp[:, :].rearrange("p (c w) -> p c w", w=CHUNK),
                axis=Axis.X)

            # L2: top-16 chunks
            m8 = sp_small.tile([128, 8], F32, tag="m8")
            m8b = sp_small.tile([128, 8], F32, tag="m8b")
            cid = sp_small.tile([128, K], U32, tag="cid")
            nc.vector.max(m8[:, :], M[:, :])
            nc.vector.max_index(cid[:, 0:8], m8[:, :], M[:, :])
            nc.vector.match_replace(M[:, :], m8[:, :], M[:, :], -3.0e38)
            nc.vector.max(m8b[:, :], M[:, :])
            nc.vector.max_index(cid[:, 8:16], m8b[:, :], M[:, :])
            cm17 = sp_small.tile([128, 1], F32, tag="cm17")
            nc.vector.match_replace(M[:, :], m8b[:, :], M[:, :], -3.0e38)
            nc.vector.reduce_max(cm17[:, :], M[:, :], axis=Axis.X)
            nc.vector.tensor_tensor(
                flg[:, 2 * t:2 * t + 1], m8b[:, 7:8], cm17[:, :], Alu.subtract)

            coff = sp_small.tile([128, K], U32, tag="coff")
            nc.vector.tensor_scalar(
                coff[:, :], cid[:, :], 4, None, Alu.logical_shift_left)
            nc.vector.tensor_tensor(coff[:, :], coff[:, :], io_rb[:, :], Alu.add)

            cand = sp_m.tile([128, K, CHUNK], F32, tag="cand")
            for w in range(K):
                nc.gpsimd.indirect_dma_start(
                    cand[:, w, :], None, s_flat,
                    IndirectOffsetOnAxis(ap=coff[:, w:w + 1], axis=1),
                    element_offset=qs * N)

            # L3: exact top-16 of 256 candidates
            cd = cand[:, :, :].rearrange("p a b -> p (a b)")
            v1 = sp_small.tile([128, 8], F32, tag="v1")
            v2 = sp_small.tile([128, 8], F32, tag="v2")
            pw = sp_small.tile([128, K], U32, tag="pw")
            nc.vector.max(v1[:, :], cd)
            nc.vector.max_index(pw[:, 0:8], v1[:, :], cd)
            nc.vector.match_replace(cd, v1[:, :], cd, -3.0e38)
            nc.vector.max(v2[:, :], cd)
            nc.vector.max_index(pw[:, 8:16], v2[:, :], cd)
            v17 = sp_small.tile([128, 1], F32, tag="v17")
            nc.vector.match_replace(cd, v2[:, :], cd, -3.0e38)
            nc.vector.reduce_max(v17[:, :], cd, axis=Axis.X)
            nc.vector.tensor_tensor(
                flg[:, 2 * t + 1:2 * t + 2], v2[:, 7:8], v17[:, :], Alu.subtract)

            # j = cid[pw>>4]<<4 | (pw&15) via DRAM round trip
            rw = sp_small.tile([128, K], U32, tag="rw")
            ow = sp_small.tile([128, K], U32, tag="ow")
            nc.vector.tensor_scalar(
                rw[:, :], pw[:, :], 4, None, Alu.logical_shift_right)
            nc.vector.tensor_scalar(
                ow[:, :], pw[:, :], 15, None, Alu.bitwise_and)
            # j_hi = cid[rw] via mask-sum (per-partition scalar broadcast)
            jhi = sp_small.tile([128, K], U32, tag="jhi")
            jtmp = sp_small.tile([128, K], U32, tag="jtmp")
            nc.vector.tensor_scalar(jhi[:, :], rw[:, :], 0, None, Alu.mult)
            for r in range(K):
                cbr = cid[:, r:r + 1].broadcast_to([128, K])
                nc.vector.scalar_tensor_tensor(
                    jtmp[:, :], rw[:, :], r, cbr,
                    Alu.is_equal, Alu.mult)
                nc.vector.tensor_tensor(
                    jhi[:, :], jhi[:, :], jtmp[:, :], Alu.add)
            j32 = sp_small.tile([128, K], U32, tag="j32")
            nc.vector.tensor_scalar(
                j32[:, :], jhi[:, :], 4, None, Alu.logical_shift_left)
            nc.vector.tensor_tensor(j32[:, :], j32[:, :], ow[:, :], Alu.bitwise_or)
            nc.sync.dma_start(j_dbg[qs:qs + 128, :], j32[:, :])
            jcs = []
            for c in range(K):
                jc = sp_small.tile([128, 1], U32, name=f"jc{c}", tag=f"jc{c}")
                nc.vector.tensor_copy(jc[:, :], j32[:, c:c + 1])
                jcs.append(jc)

            # per-slot neighbor coord gathers (one offset per partition
            # row), then PE transpose each [128,4] -> [4,128] so coords land
            # on partitions 0..2 for the w-major MLP1 matmuls.
            xjq = sp_rhs.tile([128, K * 4], F16, tag="xjq")
            for w in range(K):
                nc.gpsimd.indirect_dma_start(
                    xjq[:, w * 4:(w + 1) * 4], None, ptp4[:, :],
                    IndirectOffsetOnAxis(ap=jcs[w][:, :], axis=0),
                    element_offset=0)
            xjT = sp_rhs.tile([4, K * 128], F16, tag="xjT")
            qTh = sp_rhs.tile([3, 128], F16, tag="qTh")
            nc.scalar.copy(qTh[:, :], qT4[0:3, qs:qs + 128])
            for w in range(K):
                nc.tensor.transpose(
                    pT[:, :], xjq[:, w * 4:(w + 1) * 4], idsb[:, :])
                nc.scalar.copy(xjT[:, w * 128:(w + 1) * 128], pT[:, :])

            acc = [sp_out.tile([128, 128], F32, name=f"acc{c}",
                                tag=f"acc{c}") for c in range(3)]
            for hh in range(2):
                for w8 in range(8):
                    w = hh * 8 + w8
                    nc.tensor.matmul(
                        p1a[:, w8 * 128:(w8 + 1) * 128], w1xi[:, 0:128],
                        qTh[:, :], start=True, stop=False,
                        skip_group_check=True)
                    nc.tensor.matmul(
                        p1a[:, w8 * 128:(w8 + 1) * 128], w1xj[:, 0:128],
                        xjT[0:3, w * 128:(w + 1) * 128], start=False,
                        stop=True, skip_group_check=True)
                    nc.tensor.matmul(
                        p1b[:, w8 * 128:(w8 + 1) * 128], w1xi[:, 128:H],
                        qTh[:, :], start=True, stop=False,
                        skip_group_check=True)
                    nc.tensor.matmul(
                        p1b[:, w8 * 128:(w8 + 1) * 128], w1xj[:, 128:H],
                        xjT[0:3, w * 128:(w + 1) * 128], start=False,
                        stop=True, skip_group_check=True)
                s1a = sp_s1.tile([128, 1024], F16, tag="s1a")
                s1b = sp_s1.tile([64, 1024], F16, tag="s1b")
                nc.scalar.activation(s1a[:, :], p1a[:, :], Act.Silu,
                                     bias=b1a[:, :])
                nc.scalar.activation(s1b[:, :], p1b[:, :], Act.Silu,
                                     bias=b1b[:, :])
                for sub in range(2):
                    ss = sub * 512
                    for c in range(3):
                        p2 = p22[0]
                        if c == 0:
                            nc.tensor.matmul(
                                p2[0:2, 0:2], w2a[0:2, 0:2], w2a[0:2, 0:2],
                                start=True, stop=True, skip_group_check=True)
                        nc.tensor.matmul(
                            p2[:, :], w2a[:, c * 128:(c + 1) * 128],
                            s1a[:, ss:ss + 512], start=True, stop=False,
                            skip_group_check=True)
                        nc.tensor.matmul(
                            p2[:, :], w2b[:, c * 128:(c + 1) * 128],
                            s1b[:, ss:ss + 512], start=False, stop=True,
                            skip_group_check=True)
                        part = sp_out.tile([128, 128], F32, tag="part")
                        nc.vector.reduce_max(
                            part[:, :],
                            p2[:, :].rearrange("p (a b) -> p b a", b=128),
                            axis=Axis.X)
                        if hh == 0 and sub == 0:
                            nc.vector.tensor_copy(acc[c][:, :], part[:, :])
                        else:
                            nc.vector.tensor_tensor(
                                acc[c][:, :], acc[c][:, :], part[:, :],
                                Alu.max)
            for c in range(3):
                nc.sync.dma_start(
                    outT[c * 128:(c + 1) * 128, qs:qs + 128], acc[c][:, :])

        nc.sync.dma_start(flags[:, :], flg[:, :])

    return nc


def host_inputs_for_core(core, point, W1, b1, W2, b2):
    b = core // 2
    half = core % 2
    p = np.asarray(point[b], dtype=np.float32)
    qsl = slice(half * Q, (half + 1) * Q)
    w1a = np.asarray(W1[:3], np.float32)
    w1b = np.asarray(W1[3:], np.float32)
    w1_bot = w1b - w1a                 # pairs with xi rows
    w1_top = w1a * np.float32(0.5)     # pairs with 2*xj rows
    return {
        "ptT2": np.ascontiguousarray((2.0 * p).T.astype(np.float16)),
        "ptp4": np.ascontiguousarray(np.concatenate(
            [(2.0 * p).astype(np.float16),
             np.zeros((N, 1), np.float16)], 1)),
        "ident": np.ascontiguousarray(np.eye(128, dtype=np.float16)),
        "qrn": np.ascontiguousarray(np.concatenate([
            np.concatenate([p[qsl].T, np.ones((1, Q), np.float32)], 0),
            np.concatenate(
                [(2.0 * p).T,
                 -(p.astype(np.float32) ** 2).sum(-1)[None, :]], 0),
        ], 1).astype(np.float32)),
        "w1c": np.ascontiguousarray(
            np.concatenate([w1_bot, w1_top], 0).astype(np.float16)),
        "w2": np.ascontiguousarray(np.asarray(W2, np.float16)),
        "b1d": np.ascontiguousarray(np.asarray(b1, np.float32)[:, None]),
        "iota_rb": np.ascontiguousarray(
            (np.arange(128, dtype=np.uint32)[:, None] * np.uint32(N))
            * np.ones((1, K), np.uint32)),
        "iota16": np.ascontiguousarray(
            (np.arange(128, dtype=np.uint32)[:, None] * np.uint32(K))
            * np.ones((1, K), np.uint32)),
    }


def _host_repair(out, flags_per_core, point, W1, b1, W2, b2, k):
    """Recompute flagged (possibly tie-ambiguous) queries with
    reference-identical fp32 arithmetic."""
    f32 = np.float32
    W1 = np.asarray(W1, f32)
    b1 = np.asarray(b1, f32)
    W2 = np.asarray(W2, f32)
    b2 = np.asarray(b2, f32)
    n_repaired = 0
    for core in range(2 * B):
        b = core // 2
        half = core % 2
        fl = flags_per_core[core].reshape(128, NT, 2)
        gap = fl.min(-1)                        # [128, NT]
        pp, tt = np.nonzero(gap < TIE_EPS)
        if len(pp) == 0:
            continue
        qidx = half * Q + tt * 128 + pp
        pb = np.asarray(point[b], f32)
        diff = pb[qidx][:, None, :] - pb[None, :, :]
        dist = (diff * diff).sum(-1)
        idx = np.argsort(dist, axis=-1, kind="stable")[:, :k]
        neigh = pb[idx]
        rel = neigh - pb[qidx][:, None, :]
        ctr = np.broadcast_to(pb[qidx][:, None, :], rel.shape)
        feat = np.concatenate([rel, ctr], -1)
        h = feat @ W1 + b1
        h = h * (f32(1.0) / (f32(1.0) + np.exp(-h)))
        h2 = h @ W2 + b2
        out[b, qidx, :] = h2.max(-2)
        n_repaired += len(pp)
    return n_repaired


_CACHE = {}


def _get_program():
    if "nc" not in _CACHE:
        nc = bacc.Bacc("TRN2", debug=False, num_swdge_queues=1)
        build_program(nc)
        nc.compile()
        _CACHE["nc"] = nc
    return _CACHE["nc"]


def kernel(point, W1, b1, W2, b2, k, _trace=False):
    point = np.asarray(point, np.float32)
    k = int(k)
    assert k == K and point.shape == (B, N, 3)

    nc = _get_program()
    in_maps = [host_inputs_for_core(c, point, W1, b1, W2, b2)
               for c in range(2 * B)]
    try:
        res = run_bass_kernel_spmd(nc, in_maps, core_ids=list(range(2 * B)),
                                   trace=_trace)
    except ModuleNotFoundError:
        res = run_bass_kernel_spmd(nc, in_maps, core_ids=list(range(2 * B)),
                                   trace=False)

    out = np.empty((B, N, C), np.float32)
    flags_per_core = []
    for core in range(2 * B):
        b = core // 2
        half = core % 2
        outT = np.asarray(res.results[core]["outT"])
        out[b, half * Q:(half + 1) * Q, :] = outT.T
        flags_per_core.append(np.asarray(res.results[core]["flags"]))
    out += np.asarray(b2, np.float32)[None, None, :]

    n_rep = _host_repair(out, flags_per_core, point, W1, b1, W2, b2, k)
    if _trace:
        return out, res, n_rep
    return out

